# revision 5
# baseline (speedup 1.0000x reference)
"""MLA (multi-headed latent attention) forward on 8 Trainium2 NeuronCores.

Sharding: data-parallel over batch (4) x tensor-parallel over heads (2):
core c handles batch c//2 with heads [16*(c%2), 16*(c%2)+16).
Each core computes a partial (H-dim) output contribution; host sums the
TP pair and stacks batches.

All heavy matmuls run as float32r (TF32-like) on the PE array.
Layout is feature-major ("T" = [feature, token]) everywhere except the
fused q_a/ckv projection (token-major for the free-axis layernorms) and
v (token-major for the PV matmul).
"""

import numpy as np
import concourse.bass as bass
import concourse.mybir as mybir
import concourse.tile as tile
from concourse import bacc
from concourse import bass_utils

F32 = mybir.dt.float32
F32R = mybir.dt.float32r
AX = mybir.AxisListType
OP = mybir.AluOpType
AF = mybir.ActivationFunctionType

B, S, H, NH = 4, 1024, 4096, 32
QL, KVL, RD, ND, VD = 1536, 512, 64, 128, 128
QHD = ND + RD  # 192
EPS = 1e-6
NCORES = 8
TP = 2                 # tensor-parallel ways (heads)
HPC = NH // TP         # 16 heads per core
G = 2                  # heads per group
NG = HPC // G          # 8 groups
TOKT = S // 128        # 8 token tiles
KH = H // 128          # 32 contraction tiles for H
WA_COLS = QL + KVL + RD  # 2112
WA_BLOCKS = [(0, 512), (512, 512), (1024, 512), (1536, 512), (2048, 64)]
SCALE = float(QHD) ** -0.5

# rope feature permutation: pairs (d, d+32) land 16 lanes apart within a
# 32-partition quadrant so stream_shuffle can do rotate_half.
DIMS_PERM = np.array(
    list(range(0, 16)) + list(range(32, 48))
    + list(range(16, 32)) + list(range(48, 64)), dtype=np.int64)
SHUF_MASK = [(i + 16) % 32 for i in range(32)]

_NC_CACHE = {}


def _build_nc():
    nc = bacc.Bacc("TRN2", target_bir_lowering=False, debug=False)
    dt_in = {}

    def din(name, shape):
        dt_in[name] = nc.dram_tensor(name, shape, F32, kind="ExternalInput").ap()
        return dt_in[name]

    hsT = din("hsT", (H, S))
    wa = din("wa", (H, WA_COLS))
    wqb = din("wqb", (QL, HPC * QHD))
    wkvbk = din("wkvbk", (KVL, HPC * ND))
    wkvbv = din("wkvbv", (KVL, HPC * VD))
    wo = din("wo", (HPC * VD, H))
    csq = din("csq", (128, S))
    ssq = din("ssq", (128, S))
    gq = din("gq", (128, QL))
    gk = din("gk", (128, KVL))
    tri = din("tri", (128, 128))
    ones_in = din("ones_in", (128, 1))
    outT = nc.dram_tensor("outT", (H, S), F32, kind="ExternalOutput").ap()

    with tile.TileContext(nc) as tc:
        with tc.tile_pool(name="pers", bufs=1) as pers, \
             tc.tile_pool(name="dr", bufs=1, space="DRAM") as dr:
            # ---------------- persistent tiles ----------------
            q_anT = pers.tile([128, 12 * S], F32R)     # LN(q_a)^T  (1536, 1024)
            kv_cnT = pers.tile([128, 4 * S], F32R)     # LN(kv_c)^T (512, 1024)
            ones_t = pers.tile([128, 1], F32R)
            qac = dr.tile([S, WA_COLS], F32)           # q_a || ckv  token-major scratch
            nc.sync.dma_start(out=ones_t[:, :], in_=ones_in.bitcast(F32R))

            # ======== phase 1: q_a || ckv = hs @ [Wqa|Wkva]  (token-major, to DRAM) ========
            with tc.tile_pool(name="p1sb", bufs=1) as p1sb, \
                 tc.tile_pool(name="p1wa", bufs=3) as p1wa, \
                 tc.tile_pool(name="p1st", bufs=3) as p1st, \
                 tc.tile_pool(name="p1ps", bufs=8, space="PSUM") as p1ps:
                hsT_all = p1sb.tile([128, KH * S], F32R)   # 128KB/partition
                for k in range(KH):
                    nc.sync.dma_start(
                        out=hsT_all[:, k * S:(k + 1) * S],
                        in_=hsT[k * 128:(k + 1) * 128, :].bitcast(F32R))

                for (off, w) in WA_BLOCKS:
                    psums = [p1ps.tile([128, w], F32, tag="p1", name=f"p1_{off}_{t}") for t in range(TOKT)]
                    for k in range(KH):
                        wt = p1wa.tile([128, w], F32R, tag="wa")
                        nc.sync.dma_start(
                            out=wt[:, :], in_=wa[k * 128:(k + 1) * 128, off:off + w].bitcast(F32R))
                        for t in range(TOKT):
                            nc.tensor.matmul(
                                psums[t][:, :],
                                hsT_all[:, k * S + t * 128: k * S + (t + 1) * 128],
                                wt[:, :], start=(k == 0), stop=(k == KH - 1))
                    for t in range(TOKT):
                        stg = p1st.tile([128, w], F32, tag="p1st")
                        nc.scalar.copy(stg[:, :], psums[t][:, :])
                        nc.sync.dma_start(
                            out=qac[t * 128:(t + 1) * 128, off:off + w], in_=stg[:, :])

            # tiles spanning P1b..P3 (allocated only after phase 1 frees hsT)
            span2 = tc.tile_pool(name="span2", bufs=1)
            sp2 = span2.__enter__()
            kpeT2 = sp2.tile([128, S], F32R)           # roped k_pe^T duplicated per 64-half
            csq_t = sp2.tile([128, S], F32R)
            ssq_t = sp2.tile([128, S], F32R)
            tri_t = sp2.tile([128, 128], F32R)
            nc.sync.dma_start(out=csq_t[:, :], in_=csq.bitcast(F32R))
            nc.sync.dma_start(out=ssq_t[:, :], in_=ssq.bitcast(F32R))
            nc.sync.dma_start(out=tri_t[:, :], in_=tri.bitcast(F32R))

            # ======== phase 1b: LN (token-major) + transposes to feature-major ========
            with tc.tile_pool(name="lnsb", bufs=3) as lnsb, \
                 tc.tile_pool(name="lngb", bufs=1) as lngb, \
                 tc.tile_pool(name="lnsc", bufs=2) as lnsc, \
                 tc.tile_pool(name="lnsm", bufs=6) as lnsm, \
                 tc.tile_pool(name="tpps", bufs=4, space="PSUM") as tpps:
                gq_t = lngb.tile([128, QL], F32)
                gk_t = lngb.tile([128, KVL], F32)
                kpeT = lngb.tile([64, S], F32R)        # pre-rope k_pe^T (P1b only)
                ident = lngb.tile([128, 128], F32)
                from concourse.masks import make_identity
                make_identity(nc, ident[:, :])
                nc.sync.dma_start(out=gq_t[:, :], in_=gq)
                nc.sync.dma_start(out=gk_t[:, :], in_=gk)
                for t in range(TOKT):
                    xt = lnsb.tile([128, WA_COLS], F32, tag="ln")
                    nc.sync.dma_start(out=xt[:, :], in_=qac[t * 128:(t + 1) * 128, :])
                    for (lo, ln_w, g_t) in ((0, QL, gq_t), (QL, KVL, gk_t)):
                        x = xt[:, lo:lo + ln_w]
                        s = lnsm.tile([128, 1], F32, tag="s")
                        nc.vector.tensor_reduce(s[:, :], x, AX.X, OP.add)
                        mean = lnsm.tile([128, 1], F32, tag="mean")
                        nc.vector.tensor_scalar_mul(mean[:, :], s[:, :], 1.0 / ln_w)
                        nc.vector.tensor_scalar(
                            out=x, in0=x, scalar1=mean[:, :], scalar2=None, op0=OP.subtract)
                        sq = lnsc.tile([128, QL], F32, tag="sq")
                        ssum = lnsm.tile([128, 1], F32, tag="ssum")
                        nc.scalar.activation(sq[:, :ln_w], x, AF.Square, accum_out=ssum[:, :])
                        var = lnsm.tile([128, 1], F32, tag="var")
                        nc.vector.tensor_scalar_mul(var[:, :], ssum[:, :], 1.0 / ln_w)
                        nc.vector.tensor_scalar_add(var[:, :], var[:, :], EPS)
                        std = lnsm.tile([128, 1], F32, tag="std")
                        nc.scalar.activation(std[:, :], var[:, :], AF.Sqrt)
                        rstd = lnsm.tile([128, 1], F32, tag="rstd")
                        nc.vector.reciprocal(rstd[:, :], std[:, :])
                        nc.vector.scalar_tensor_tensor(
                            out=x, in0=x, scalar=rstd[:, :], in1=g_t[:, :ln_w],
                            op0=OP.mult, op1=OP.mult)
                    # transposes -> feature-major
                    for m in range(12):
                        pt = tpps.tile([128, 128], F32, tag="tp")
                        nc.tensor.transpose(pt[:, :], xt[:, m * 128:(m + 1) * 128], ident[:, :])
                        nc.scalar.copy(q_anT[:, m * S + t * 128: m * S + (t + 1) * 128], pt[:, :])
                    for m in range(4):
                        pt = tpps.tile([128, 128], F32, tag="tp")
                        nc.tensor.transpose(pt[:, :], xt[:, QL + m * 128: QL + (m + 1) * 128], ident[:, :])
                        nc.scalar.copy(kv_cnT[:, m * S + t * 128: m * S + (t + 1) * 128], pt[:, :])
                    pt = tpps.tile([128, 128], F32, tag="tp")
                    nc.tensor.transpose(pt[:64, :], xt[:, QL + KVL: QL + KVL + 64], ident[:, :])
                    nc.scalar.copy(kpeT[:, t * 128:(t + 1) * 128], pt[:64, :])

                # rope k_pe (shared across heads), then duplicate to both halves
                kp_sh = lnsc.tile([64, S], F32R, tag="kpsh")
                nc.vector.stream_shuffle(
                    kp_sh[:, :].bitcast(F32), kpeT[:, :].bitcast(F32), SHUF_MASK)
                nc.vector.tensor_tensor(out=kp_sh[:, :], in0=kp_sh[:, :], in1=ssq_t[:64, :], op=OP.mult)
                nc.vector.tensor_tensor(out=kpeT[:, :], in0=kpeT[:, :], in1=csq_t[:64, :], op=OP.mult)
                nc.vector.tensor_tensor(out=kpeT[:, :], in0=kpeT[:, :], in1=kp_sh[:, :], op=OP.add)
                nc.sync.dma_start(out=kpeT2[0:64, :], in_=kpeT[:, :])
                nc.sync.dma_start(out=kpeT2[64:128, :], in_=kpeT[:, :])

            # ======== phase 2 + 3 ========
            with tc.tile_pool(name="otp", bufs=1) as otp:
                oT = otp.tile([128, HPC * S], F32R)    # normalized o^T (2048, 1024)
                with tc.tile_pool(name="gq2", bufs=1) as gqp, \
                     tc.tile_pool(name="gkv", bufs=1) as gkvp, \
                     tc.tile_pool(name="wq", bufs=14) as wqp, \
                     tc.tile_pool(name="wk", bufs=6) as wkp, \
                     tc.tile_pool(name="wv", bufs=5) as wvp, \
                     tc.tile_pool(name="rshp", bufs=1) as rshp, \
                     tc.tile_pool(name="pp", bufs=4) as ppool, \
                     tc.tile_pool(name="rsp", bufs=1) as rsp, \
                     tc.tile_pool(name="rbp", bufs=1) as rbp, \
                     tc.tile_pool(name="pjps", bufs=2, space="PSUM") as pjps, \
                     tc.tile_pool(name="sps", bufs=2, space="PSUM") as sps, \
                     tc.tile_pool(name="ops", bufs=2, space="PSUM") as ops, \
                     tc.tile_pool(name="smps", bufs=2, space="PSUM") as smps:
                    for g in range(NG):
                        # ---- q^T for this group: 3 m-tiles (2x nope, 1x pe pair) ----
                        qT = gqp.tile([128, 3 * S], F32R, tag="qT")
                        for m in range(3):
                            wts = []
                            for k in range(12):
                                wt = wqp.tile([128, 128], F32R, tag="wqb", name=f"wqb_{g}_{m}_{k}")
                                nc.sync.dma_start(
                                    out=wt[:, :],
                                    in_=wqb[k * 128:(k + 1) * 128,
                                            g * 384 + m * 128: g * 384 + (m + 1) * 128].bitcast(F32R))
                                wts.append(wt)
                            for qh in range(2):
                                ps = pjps.tile([128, 512], F32, tag="pj")
                                for k in range(12):
                                    nc.tensor.matmul(
                                        ps[:, :], wts[k][:, :],
                                        q_anT[:, k * S + qh * 512: k * S + qh * 512 + 512],
                                        start=(k == 0), stop=(k == 11))
                                nc.scalar.copy(qT[:, m * S + qh * 512: m * S + qh * 512 + 512], ps[:, :])
                        # rope the pe tile (m=2): rows 0:64 = head0 pe, 64:128 = head1 pe
                        pe = qT[:, 2 * S:3 * S]
                        rsh = rshp.tile([128, S], F32R, tag="rsh")
                        nc.vector.stream_shuffle(rsh[:, :].bitcast(F32), pe.bitcast(F32), SHUF_MASK)
                        nc.vector.tensor_tensor(out=rsh[:, :], in0=rsh[:, :], in1=ssq_t[:, :], op=OP.mult)
                        nc.vector.tensor_tensor(out=pe, in0=pe, in1=csq_t[:, :], op=OP.mult)
                        nc.vector.tensor_tensor(out=pe, in0=pe, in1=rsh[:, :], op=OP.add)

                        # ---- k_nope^T: 2 m-tiles ----
                        knT = gkvp.tile([128, 2 * S], F32R, tag="knT")
                        for m in range(2):
                            wts = []
                            for k in range(4):
                                wt = wkp.tile([128, 128], F32R, tag="wk", name=f"wk_{g}_{m}_{k}")
                                nc.sync.dma_start(
                                    out=wt[:, :],
                                    in_=wkvbk[k * 128:(k + 1) * 128,
                                              g * 256 + m * 128: g * 256 + (m + 1) * 128].bitcast(F32R))
                                wts.append(wt)
                            for qh in range(2):
                                ps = pjps.tile([128, 512], F32, tag="pj")
                                for k in range(4):
                                    nc.tensor.matmul(
                                        ps[:, :], wts[k][:, :],
                                        kv_cnT[:, k * S + qh * 512: k * S + qh * 512 + 512],
                                        start=(k == 0), stop=(k == 3))
                                nc.scalar.copy(knT[:, m * S + qh * 512: m * S + qh * 512 + 512], ps[:, :])

                        # ---- v token-major: (128 tok, 8 toktile x 256 cols) ----
                        v_sb = gkvp.tile([128, TOKT * G * VD], F32R, tag="v")
                        wvs = []
                        for k in range(4):
                            wt = wvp.tile([128, 256], F32R, tag="wv", name=f"wv_{g}_{k}")
                            nc.sync.dma_start(
                                out=wt[:, :],
                                in_=wkvbv[k * 128:(k + 1) * 128, g * 256:(g + 1) * 256].bitcast(F32R))
                            wvs.append(wt)
                        for t in range(TOKT):
                            ps = pjps.tile([128, 512], F32, tag="pj")
                            for k in range(4):
                                nc.tensor.matmul(
                                    ps[:, :256], kv_cnT[:, k * S + t * 128: k * S + (t + 1) * 128],
                                    wvs[k][:, :], start=(k == 0), stop=(k == 3))
                            nc.scalar.copy(v_sb[:, t * 256:(t + 1) * 256], ps[:, :256])

                        # ---- attention per head ----
                        for hh in range(G):
                            hg = g * G + hh
                            po = [ops.tile([128, 512], F32, tag="po", name=f"po_{hg}_{qh}") for qh in range(2)]
                            psm = [smps.tile([1, 512], F32, tag="psm", name=f"psm_{hg}_{qh}") for qh in range(2)]
                            for ik in range(TOKT):
                                qstart = 128 * ik
                                for qh in range(2):
                                    lo = max(qstart, 512 * qh)
                                    hi = 512 * (qh + 1)
                                    if lo >= hi:
                                        continue
                                    w = hi - lo
                                    ps_s = sps.tile([128, 512], F32, tag="ps")
                                    nc.tensor.matmul(
                                        ps_s[:, :w],
                                        knT[:, hh * S + ik * 128: hh * S + (ik + 1) * 128],
                                        qT[:, hh * S + lo: hh * S + hi],
                                        start=True, stop=False)
                                    nc.tensor.matmul(
                                        ps_s[:, :w],
                                        kpeT2[hh * 64:(hh + 1) * 64, ik * 128:(ik + 1) * 128],
                                        qT[hh * 64:(hh + 1) * 64, 2 * S + lo: 2 * S + hi],
                                        start=False, stop=True)
                                    p = ppool.tile([128, 512], F32R, tag="p")
                                    nc.scalar.activation(p[:, :w], ps_s[:, :w], AF.Exp, scale=SCALE)
                                    if lo == qstart:
                                        nc.vector.tensor_tensor(
                                            out=p[:, 0:128], in0=p[:, 0:128], in1=tri_t[:, :], op=OP.mult)
                                    last_ik = 3 if qh == 0 else 7
                                    nc.tensor.matmul(
                                        psm[qh][:, lo - 512 * qh: hi - 512 * qh],
                                        ones_t[:, :], p[:, :w],
                                        start=(ik == 0), stop=(ik == last_ik))
                                    nc.tensor.matmul(
                                        po[qh][:, lo - 512 * qh: hi - 512 * qh],
                                        v_sb[:, ik * 256 + hh * 128: ik * 256 + (hh + 1) * 128],
                                        p[:, :w],
                                        start=(ik == 0), stop=(ik == last_ik))
                            rs = rsp.tile([1, S], F32, tag="rs")
                            nc.vector.reciprocal(rs[:, 0:512], psm[0][:, :])
                            nc.vector.reciprocal(rs[:, 512:1024], psm[1][:, :])
                            rb = rbp.tile([128, S], F32, tag="rb")
                            nc.gpsimd.partition_broadcast(rb[:, :], rs[:, :])
                            for qh in range(2):
                                nc.vector.tensor_tensor(
                                    out=oT[:, hg * S + qh * 512: hg * S + qh * 512 + 512],
                                    in0=po[qh][:, :], in1=rb[:, qh * 512: qh * 512 + 512], op=OP.mult)

                # ======== phase 3: out^T = Wo^T @ o ========
                with tc.tile_pool(name="wop", bufs=20) as wop, \
                     tc.tile_pool(name="op", bufs=3) as outp, \
                     tc.tile_pool(name="wops", bufs=2, space="PSUM") as wops:
                    for hr in range(H // 128):
                        wts = []
                        for m in range(HPC * VD // 128):
                            wt = wop.tile([128, 128], F32R, tag="wo", name=f"wo_{hr}_{m}")
                            nc.sync.dma_start(
                                out=wt[:, :],
                                in_=wo[m * 128:(m + 1) * 128, hr * 128:(hr + 1) * 128].bitcast(F32R))
                            wts.append(wt)
                        for qh in range(2):
                            ps = wops.tile([128, 512], F32, tag="pw")
                            for m in range(HPC * VD // 128):
                                nc.tensor.matmul(
                                    ps[:, :], wts[m][:, :],
                                    oT[:, m * S + qh * 512: m * S + qh * 512 + 512],
                                    start=(m == 0), stop=(m == HPC * VD // 128 - 1))
                            ot = outp.tile([128, 512], F32, tag="out")
                            nc.scalar.copy(ot[:, :], ps[:, :])
                            nc.sync.dma_start(
                                out=outT[hr * 128:(hr + 1) * 128, qh * 512:(qh + 1) * 512],
                                in_=ot[:, :])
            span2.__exit__(None, None, None)
    nc.compile()
    return nc


def _host_prep(inputs):
    hs = np.asarray(inputs["hidden_states"], np.float32)
    cos = np.asarray(inputs["cos"], np.float32)
    sin = np.asarray(inputs["sin"], np.float32)
    pid = np.asarray(inputs["position_ids"]).astype(np.int64)
    Wqa = np.asarray(inputs["Wqa"], np.float32)
    gqa = np.asarray(inputs["gqa"], np.float32)
    Wqb = np.asarray(inputs["Wqb"], np.float32)
    Wkva = np.asarray(inputs["Wkva"], np.float32)
    gkva = np.asarray(inputs["gkva"], np.float32)
    Wkvb = np.asarray(inputs["Wkvb"], np.float32)
    Wo = np.asarray(inputs["Wo"], np.float32)

    # Wa = [Wqa | Wkva(kv) | Wkva(pe, rope-permuted)]
    wa = np.concatenate([Wqa, Wkva[:, :KVL], Wkva[:, KVL:][:, DIMS_PERM]], axis=1)
    wa = np.ascontiguousarray(wa)

    # sign pattern for the shuffle-based rotate_half
    sign = np.where(DIMS_PERM < RD // 2, -1.0, 1.0).astype(np.float32)[:, None]

    tri = np.zeros((128, 128), np.float32)
    kp, q = np.mgrid[0:128, 0:128]
    tri[q >= kp] = 1.0

    per_core = []
    w4 = Wqb.reshape(QL, NH, QHD)
    wk4 = Wkvb.reshape(KVL, NH, ND + VD)
    for c in range(NCORES):
        b, t = divmod(c, TP)
        heads = slice(t * HPC, (t + 1) * HPC)
        # Wqb: group-blocked [h0 nope | h1 nope | h0 pe' | h1 pe'] per group
        wq = w4[:, heads]                       # (QL, 16, 192)
        nope = wq[:, :, :ND]                    # (QL, 16, 128)
        pe = wq[:, :, ND:][:, :, DIMS_PERM]     # (QL, 16, 64) permuted
        blocks = []
        for g in range(NG):
            blocks.extend([nope[:, 2 * g], nope[:, 2 * g + 1], pe[:, 2 * g], pe[:, 2 * g + 1]])
        wqb_c = np.ascontiguousarray(np.concatenate(blocks, axis=1))

        wkc = wk4[:, heads]
        wkvbk_c = np.ascontiguousarray(wkc[:, :, :ND].reshape(KVL, HPC * ND))
        wkvbv_c = np.ascontiguousarray(wkc[:, :, ND:].reshape(KVL, HPC * VD))
        wo_c = np.ascontiguousarray(Wo[t * HPC * VD:(t + 1) * HPC * VD])

        cos_g = cos[pid[b]]                     # (S, RD)
        sin_g = sin[pid[b]]
        cosT = np.ascontiguousarray(cos_g.T[DIMS_PERM])   # (64, S)
        sinT = np.ascontiguousarray(sin_g.T[DIMS_PERM])
        csq = np.ascontiguousarray(np.vstack([cosT, cosT]))
        ssq = np.ascontiguousarray(np.vstack([sinT * sign, sinT * sign]))

        per_core.append({
            "hsT": np.ascontiguousarray(hs[b].T),
            "wa": wa,
            "wqb": wqb_c,
            "wkvbk": wkvbk_c,
            "wkvbv": wkvbv_c,
            "wo": wo_c,
            "csq": csq,
            "ssq": ssq,
            "gq": np.ascontiguousarray(np.broadcast_to(gqa, (128, QL))),
            "gk": np.ascontiguousarray(np.broadcast_to(gkva, (128, KVL))),
            "tri": tri,
            "ones_in": np.ones((128, 1), np.float32),
        })
    return per_core


def kernel(**inputs):
    if "nc" not in _NC_CACHE:
        _NC_CACHE["nc"] = _build_nc()
    nc = _NC_CACHE["nc"]
    in_maps = _host_prep(inputs)
    res = bass_utils.run_bass_kernel_spmd(nc, in_maps, core_ids=list(range(NCORES)))
    outs = []
    for b in range(B):
        acc = res.results[TP * b]["outT"].astype(np.float32)
        for t in range(1, TP):
            acc = acc + res.results[TP * b + t]["outT"]
        outs.append(acc.T)
    return np.stack(outs, axis=0)


# revision 10
# speedup vs baseline: 1.4000x; 1.4000x over previous
"""MLA (multi-headed latent attention) forward on 8 Trainium2 NeuronCores.

Sharding: data-parallel over batch (4) x tensor-parallel over heads (2):
core c handles batch c//2 with heads [16*(c%2), 16*(c%2)+16).
Each core computes a partial (H-dim) output contribution; host sums the
TP pair and stacks batches.

All heavy matmuls run as float32r (TF32-like) on the PE array.
Layout is feature-major ("T" = [feature, token]) everywhere except the
fused q_a/ckv projection (token-major for the free-axis layernorms) and
v (token-major for the PV matmul).
"""

import numpy as np
import concourse.bass as bass
import concourse.mybir as mybir
import concourse.tile as tile
from concourse import bacc
from concourse import bass_utils

F32 = mybir.dt.float32
F32R = mybir.dt.float32r
AX = mybir.AxisListType
OP = mybir.AluOpType
AF = mybir.ActivationFunctionType

B, S, H, NH = 4, 1024, 4096, 32
QL, KVL, RD, ND, VD = 1536, 512, 64, 128, 128
QHD = ND + RD  # 192
EPS = 1e-6
NCORES = 8
TP = 2                 # tensor-parallel ways (heads)
HPC = NH // TP         # 16 heads per core
G = 2                  # heads per group
NG = HPC // G          # 8 groups
TOKT = S // 128        # 8 token tiles
KH = H // 128          # 32 contraction tiles for H
WA_COLS = QL + KVL + RD  # 2112
WA_BLOCKS = [(0, 512), (512, 512), (1024, 512), (1536, 512), (2048, 64)]
SCALE = float(QHD) ** -0.5

# rope feature permutation: pairs (d, d+32) land 16 lanes apart within a
# 32-partition quadrant so stream_shuffle can do rotate_half.
DIMS_PERM = np.array(
    list(range(0, 16)) + list(range(32, 48))
    + list(range(16, 32)) + list(range(48, 64)), dtype=np.int64)
SHUF_MASK = [(i + 16) % 32 for i in range(32)]

_NC_CACHE = {}


def _build_nc():
    nc = bacc.Bacc("TRN2", target_bir_lowering=False, debug=False)
    dt_in = {}

    def din(name, shape):
        dt_in[name] = nc.dram_tensor(name, shape, F32, kind="ExternalInput").ap()
        return dt_in[name]

    hsT = din("hsT", (H, S))
    wa = din("wa", (H, WA_COLS))
    wqb = din("wqb", (QL, HPC * QHD))
    wkvbk = din("wkvbk", (KVL, HPC * ND))
    wkvbv = din("wkvbv", (KVL, HPC * VD))
    wo = din("wo", (HPC * VD, H))
    csq = din("csq", (128, S))
    ssq = din("ssq", (128, S))
    tri = din("tri", (128, 128))
    ones_in = din("ones_in", (128, 1))
    outT = nc.dram_tensor("outT", (H, S), F32, kind="ExternalOutput").ap()

    with tile.TileContext(nc) as tc:
        with tc.tile_pool(name="pers", bufs=1) as pers, \
             tc.tile_pool(name="dr", bufs=1, space="DRAM") as dr:
            # ---------------- persistent tiles ----------------
            q_anT = pers.tile([128, 12 * S], F32R)     # LN(q_a)^T  (1536, 1024)
            kv_cnT = pers.tile([128, 4 * S], F32R)     # LN(kv_c)^T (512, 1024)
            ones_t = pers.tile([128, 1], F32R)
            nc.sync.dma_start(out=ones_t[:, :], in_=ones_in.bitcast(F32R))

            # tiles spanning P1..P3
            span2 = tc.tile_pool(name="span2", bufs=1)
            sp2 = span2.__enter__()
            kpeT2 = sp2.tile([128, S], F32R)           # roped k_pe^T on both 64-halves
            csq_t = sp2.tile([128, S], F32R)
            ssq_t = sp2.tile([128, S], F32R)
            tri_t = sp2.tile([128, 128], F32R)
            nc.sync.dma_start(out=csq_t[:, :], in_=csq.bitcast(F32R))
            nc.sync.dma_start(out=ssq_t[:, :], in_=ssq.bitcast(F32R))
            nc.sync.dma_start(out=tri_t[:, :], in_=tri.bitcast(F32R))

            # ======== phase 1 (feature-major): X^T = Wa^T @ hs^T, LN fused ========
            # m-tiles: 12x q_a -> q_anT, 4x kv_c -> kv_cnT, 1x k_pe(64) -> kpeT2[0:64]
            M_TILES = ([("qa", i) for i in range(12)] + [("kv", i) for i in range(4)]
                       + [("pe", 0)])
            NQ = 4  # H-contraction quarters (8 k-subtiles each)

            def dest_of(kind, mi):
                if kind == "qa":
                    return q_anT[:, mi * S:(mi + 1) * S], 128
                if kind == "kv":
                    return kv_cnT[:, mi * S:(mi + 1) * S], 128
                return kpeT2[0:64, :], 64

            with tc.tile_pool(name="hsp", bufs=2) as hsp, \
                 tc.tile_pool(name="p1wa", bufs=3) as p1wa, \
                 tc.tile_pool(name="sqp", bufs=2) as sqp, \
                 tc.tile_pool(name="rowp", bufs=1) as rowp, \
                 tc.tile_pool(name="bcp", bufs=1) as bcp, \
                 tc.tile_pool(name="p1ps", bufs=3, space="PSUM") as p1ps, \
                 tc.tile_pool(name="stps", bufs=4, space="PSUM") as stps:
                for quarter in range(NQ):
                    hsq = hsp.tile([128, 8, S], F32R, tag="hsq", name=f"hsq_{quarter}")
                    nc.sync.dma_start(
                        out=hsq[:, :, :],
                        in_=hsT[quarter * 1024:(quarter + 1) * 1024, :]
                            .rearrange("(k p) t -> p k t", p=128).bitcast(F32R))
                    for (kind, mi) in M_TILES:
                        coff = {"qa": 0, "kv": QL, "pe": QL + KVL}[kind] + mi * 128
                        wcols = 64 if kind == "pe" else 128
                        wt = p1wa.tile([128, 8, 128], F32R, tag="wa", name=f"wa_{quarter}_{kind}_{mi}")
                        nc.sync.dma_start(
                            out=wt[:, :, :wcols],
                            in_=wa[quarter * 1024:(quarter + 1) * 1024, coff:coff + wcols]
                                .rearrange("(k p) c -> p k c", p=128).bitcast(F32R))
                        dest, rows = dest_of(kind, mi)
                        for qh in range(2):
                            ps = p1ps.tile([128, 512], F32, tag="p1")
                            for k in range(8):
                                nc.tensor.matmul(
                                    ps[:rows, :], wt[:, k, :wcols],
                                    hsq[:, k, qh * 512: qh * 512 + 512],
                                    start=(k == 0), stop=(k == 7))
                            dslice = dest[:, qh * 512: qh * 512 + 512]
                            if quarter == 0:
                                nc.scalar.copy(dslice, ps[:rows, :])
                            else:
                                nc.vector.tensor_tensor(
                                    out=dslice, in0=dslice, in1=ps[:rows, :], op=OP.add)

                # ---- LN stats via ones-matmuls (partition reduction), then apply ----
                for kind, nmt, n_feat, destT in (("qa", 12, QL, q_anT), ("kv", 4, KVL, kv_cnT)):
                    psum_s = [stps.tile([1, 512], F32, tag="st", name=f"st_{kind}_{qh}") for qh in range(2)]
                    psum_q = [stps.tile([1, 512], F32, tag="st", name=f"stq_{kind}_{qh}") for qh in range(2)]
                    for mi in range(nmt):
                        sqt = sqp.tile([128, S], F32R, tag="sq")
                        nc.scalar.activation(sqt[:, :], destT[:, mi * S:(mi + 1) * S], AF.Square)
                        for qh in range(2):
                            nc.tensor.matmul(
                                psum_s[qh][:, :], ones_t[:, :],
                                destT[:, mi * S + qh * 512: mi * S + qh * 512 + 512],
                                start=(mi == 0), stop=(mi == nmt - 1))
                            nc.tensor.matmul(
                                psum_q[qh][:, :], ones_t[:, :],
                                sqt[:, qh * 512: qh * 512 + 512],
                                start=(mi == 0), stop=(mi == nmt - 1))
                    mrow = rowp.tile([1, S], F32, name=f"mrow_{kind}")
                    vrow = rowp.tile([1, S], F32, name=f"vrow_{kind}")
                    srow = rowp.tile([1, S], F32, name=f"srow_{kind}")
                    rrow = rowp.tile([1, S], F32, name=f"rrow_{kind}")
                    for qh in range(2):
                        sl = slice(qh * 512, qh * 512 + 512)
                        nc.vector.tensor_scalar_mul(mrow[:, sl], psum_s[qh][:, :], 1.0 / n_feat)
                        nc.vector.tensor_scalar_mul(vrow[:, sl], psum_q[qh][:, :], 1.0 / n_feat)
                    # var = E[x^2] - mean^2 + eps ; rstd = 1/sqrt(var)
                    nc.vector.tensor_tensor(out=srow[:, :], in0=mrow[:, :], in1=mrow[:, :], op=OP.mult)
                    nc.vector.tensor_tensor(out=vrow[:, :], in0=vrow[:, :], in1=srow[:, :], op=OP.subtract)
                    nc.vector.tensor_scalar_add(vrow[:, :], vrow[:, :], EPS)
                    nc.scalar.activation(srow[:, :], vrow[:, :], AF.Sqrt)
                    nc.vector.reciprocal(rrow[:, :], srow[:, :])
                    mb = bcp.tile([128, S], F32, name=f"mb_{kind}")
                    rb_ = bcp.tile([128, S], F32, name=f"rb_{kind}")
                    nc.gpsimd.partition_broadcast(mb[:, :], mrow[:, :])
                    nc.gpsimd.partition_broadcast(rb_[:, :], rrow[:, :])
                    for mi in range(nmt):
                        dsl = destT[:, mi * S:(mi + 1) * S]
                        nc.vector.tensor_tensor(out=dsl, in0=dsl, in1=mb[:, :], op=OP.subtract)
                        nc.vector.tensor_tensor(out=dsl, in0=dsl, in1=rb_[:, :], op=OP.mult)

                # ---- rope k_pe in place on kpeT2[0:64], then duplicate ----
                kp_sh = sqp.tile([64, S], F32R, tag="kpsh")
                nc.vector.stream_shuffle(
                    kp_sh[:, :].bitcast(F32), kpeT2[0:64, :].bitcast(F32), SHUF_MASK)
                nc.vector.tensor_tensor(out=kp_sh[:, :], in0=kp_sh[:, :], in1=ssq_t[:64, :], op=OP.mult)
                nc.vector.tensor_tensor(out=kpeT2[0:64, :], in0=kpeT2[0:64, :], in1=csq_t[:64, :], op=OP.mult)
                nc.vector.tensor_tensor(out=kpeT2[0:64, :], in0=kpeT2[0:64, :], in1=kp_sh[:, :], op=OP.add)
                nc.sync.dma_start(out=kpeT2[64:128, :], in_=kpeT2[0:64, :])

            # ======== phase 2 + 3 ========
            with tc.tile_pool(name="otp", bufs=1) as otp:
                oT = otp.tile([128, HPC * S], F32R)    # normalized o^T (2048, 1024)
                with tc.tile_pool(name="gq2", bufs=1) as gqp, \
                     tc.tile_pool(name="gkv", bufs=1) as gkvp, \
                     tc.tile_pool(name="wq", bufs=2) as wqp, \
                     tc.tile_pool(name="wk", bufs=2) as wkp, \
                     tc.tile_pool(name="wv", bufs=1) as wvp, \
                     tc.tile_pool(name="rshp", bufs=1) as rshp, \
                     tc.tile_pool(name="pp", bufs=3) as ppool, \
                     tc.tile_pool(name="rsp", bufs=1) as rsp, \
                     tc.tile_pool(name="rbp", bufs=1) as rbp, \
                     tc.tile_pool(name="pjps", bufs=2, space="PSUM") as pjps, \
                     tc.tile_pool(name="sps", bufs=2, space="PSUM") as sps, \
                     tc.tile_pool(name="ops", bufs=2, space="PSUM") as ops, \
                     tc.tile_pool(name="smps", bufs=2, space="PSUM") as smps:
                    for g in range(NG):
                        # ---- q^T for this group: 3 m-tiles (2x nope, 1x pe pair) ----
                        qT = gqp.tile([128, 3 * S], F32R, tag="qT")
                        for m in range(3):
                            wt = wqp.tile([128, 12, 128], F32R, tag="wqb", name=f"wqb_{g}_{m}")
                            nc.sync.dma_start(
                                out=wt[:, :, :],
                                in_=wqb[:, g * 384 + m * 128: g * 384 + (m + 1) * 128]
                                    .rearrange("(k p) c -> p k c", p=128).bitcast(F32R))
                            for qh in range(2):
                                ps = pjps.tile([128, 512], F32, tag="pj")
                                for k in range(12):
                                    nc.tensor.matmul(
                                        ps[:, :], wt[:, k, :],
                                        q_anT[:, k * S + qh * 512: k * S + qh * 512 + 512],
                                        start=(k == 0), stop=(k == 11))
                                nc.scalar.copy(qT[:, m * S + qh * 512: m * S + qh * 512 + 512], ps[:, :])
                        # rope the pe tile (m=2): rows 0:64 = head0 pe, 64:128 = head1 pe
                        pe = qT[:, 2 * S:3 * S]
                        rsh = rshp.tile([128, S], F32R, tag="rsh")
                        nc.vector.stream_shuffle(rsh[:, :].bitcast(F32), pe.bitcast(F32), SHUF_MASK)
                        nc.vector.tensor_tensor(out=rsh[:, :], in0=rsh[:, :], in1=ssq_t[:, :], op=OP.mult)
                        nc.vector.tensor_tensor(out=pe, in0=pe, in1=csq_t[:, :], op=OP.mult)
                        nc.vector.tensor_tensor(out=pe, in0=pe, in1=rsh[:, :], op=OP.add)

                        # ---- k_nope^T: 2 m-tiles ----
                        knT = gkvp.tile([128, 2 * S], F32R, tag="knT")
                        for m in range(2):
                            wt = wkp.tile([128, 4, 128], F32R, tag="wk", name=f"wk_{g}_{m}")
                            nc.sync.dma_start(
                                out=wt[:, :, :],
                                in_=wkvbk[:, g * 256 + m * 128: g * 256 + (m + 1) * 128]
                                    .rearrange("(k p) c -> p k c", p=128).bitcast(F32R))
                            for qh in range(2):
                                ps = pjps.tile([128, 512], F32, tag="pj")
                                for k in range(4):
                                    nc.tensor.matmul(
                                        ps[:, :], wt[:, k, :],
                                        kv_cnT[:, k * S + qh * 512: k * S + qh * 512 + 512],
                                        start=(k == 0), stop=(k == 3))
                                nc.scalar.copy(knT[:, m * S + qh * 512: m * S + qh * 512 + 512], ps[:, :])

                        # ---- v token-major: (128 tok, 8 toktile x 256 cols) ----
                        v_sb = gkvp.tile([128, TOKT * G * VD], F32R, tag="v")
                        wv_t = wvp.tile([128, 4, 256], F32R, tag="wv", name=f"wv_{g}")
                        nc.sync.dma_start(
                            out=wv_t[:, :, :],
                            in_=wkvbv[:, g * 256:(g + 1) * 256]
                                .rearrange("(k p) c -> p k c", p=128).bitcast(F32R))
                        for t in range(TOKT):
                            ps = pjps.tile([128, 512], F32, tag="pj")
                            for k in range(4):
                                nc.tensor.matmul(
                                    ps[:, :256], kv_cnT[:, k * S + t * 128: k * S + (t + 1) * 128],
                                    wv_t[:, k, :], start=(k == 0), stop=(k == 3))
                            nc.scalar.copy(v_sb[:, t * 256:(t + 1) * 256], ps[:, :256])

                        # ---- attention per head ----
                        for hh in range(G):
                            hg = g * G + hh
                            po = [ops.tile([128, 512], F32, tag="po", name=f"po_{hg}_{qh}") for qh in range(2)]
                            psm = [smps.tile([1, 512], F32, tag="psm", name=f"psm_{hg}_{qh}") for qh in range(2)]
                            for ik in range(TOKT):
                                qstart = 128 * ik
                                for qh in range(2):
                                    lo = max(qstart, 512 * qh)
                                    hi = 512 * (qh + 1)
                                    if lo >= hi:
                                        continue
                                    w = hi - lo
                                    ps_s = sps.tile([128, 512], F32, tag="ps")
                                    nc.tensor.matmul(
                                        ps_s[:, :w],
                                        knT[:, hh * S + ik * 128: hh * S + (ik + 1) * 128],
                                        qT[:, hh * S + lo: hh * S + hi],
                                        start=True, stop=False)
                                    nc.tensor.matmul(
                                        ps_s[:, :w],
                                        kpeT2[hh * 64:(hh + 1) * 64, ik * 128:(ik + 1) * 128],
                                        qT[hh * 64:(hh + 1) * 64, 2 * S + lo: 2 * S + hi],
                                        start=False, stop=True)
                                    p = ppool.tile([128, 512], F32R, tag="p")
                                    nc.scalar.activation(p[:, :w], ps_s[:, :w], AF.Exp, scale=SCALE)
                                    if lo == qstart:
                                        nc.vector.tensor_tensor(
                                            out=p[:, 0:128], in0=p[:, 0:128], in1=tri_t[:, :], op=OP.mult)
                                    last_ik = 3 if qh == 0 else 7
                                    nc.tensor.matmul(
                                        psm[qh][:, lo - 512 * qh: hi - 512 * qh],
                                        ones_t[:, :], p[:, :w],
                                        start=(ik == 0), stop=(ik == last_ik))
                                    nc.tensor.matmul(
                                        po[qh][:, lo - 512 * qh: hi - 512 * qh],
                                        v_sb[:, ik * 256 + hh * 128: ik * 256 + (hh + 1) * 128],
                                        p[:, :w],
                                        start=(ik == 0), stop=(ik == last_ik))
                            rs = rsp.tile([1, S], F32, tag="rs")
                            nc.vector.reciprocal(rs[:, 0:512], psm[0][:, :])
                            nc.vector.reciprocal(rs[:, 512:1024], psm[1][:, :])
                            rb = rbp.tile([128, S], F32, tag="rb")
                            nc.gpsimd.partition_broadcast(rb[:, :], rs[:, :])
                            for qh in range(2):
                                nc.vector.tensor_tensor(
                                    out=oT[:, hg * S + qh * 512: hg * S + qh * 512 + 512],
                                    in0=po[qh][:, :], in1=rb[:, qh * 512: qh * 512 + 512], op=OP.mult)

                # ======== phase 3: out^T = Wo^T @ o ========
                with tc.tile_pool(name="wop", bufs=3) as wop, \
                     tc.tile_pool(name="op", bufs=3) as outp, \
                     tc.tile_pool(name="wops", bufs=2, space="PSUM") as wops:
                    for hr in range(H // 128):
                        wt = wop.tile([128, 16, 128], F32R, tag="wo", name=f"wo_{hr}")
                        nc.sync.dma_start(
                            out=wt[:, :, :],
                            in_=wo[:, hr * 128:(hr + 1) * 128]
                                .rearrange("(m p) c -> p m c", p=128).bitcast(F32R))
                        for qh in range(2):
                            ps = wops.tile([128, 512], F32, tag="pw")
                            for m in range(HPC * VD // 128):
                                nc.tensor.matmul(
                                    ps[:, :], wt[:, m, :],
                                    oT[:, m * S + qh * 512: m * S + qh * 512 + 512],
                                    start=(m == 0), stop=(m == HPC * VD // 128 - 1))
                            ot = outp.tile([128, 512], F32, tag="out")
                            nc.scalar.copy(ot[:, :], ps[:, :])
                            nc.sync.dma_start(
                                out=outT[hr * 128:(hr + 1) * 128, qh * 512:(qh + 1) * 512],
                                in_=ot[:, :])
            span2.__exit__(None, None, None)
    nc.compile()
    return nc


def _host_prep(inputs):
    hs = np.asarray(inputs["hidden_states"], np.float32)
    cos = np.asarray(inputs["cos"], np.float32)
    sin = np.asarray(inputs["sin"], np.float32)
    pid = np.asarray(inputs["position_ids"]).astype(np.int64)
    Wqa = np.asarray(inputs["Wqa"], np.float32)
    gqa = np.asarray(inputs["gqa"], np.float32)
    Wqb = np.asarray(inputs["Wqb"], np.float32)
    Wkva = np.asarray(inputs["Wkva"], np.float32)
    gkva = np.asarray(inputs["gkva"], np.float32)
    Wkvb = np.asarray(inputs["Wkvb"], np.float32)
    Wo = np.asarray(inputs["Wo"], np.float32)

    # Wa = [Wqa | Wkva(kv) | Wkva(pe, rope-permuted)]
    wa = np.concatenate([Wqa, Wkva[:, :KVL], Wkva[:, KVL:][:, DIMS_PERM]], axis=1)
    wa = np.ascontiguousarray(wa)
    # fold LN gains into the B-projections (bias terms are zero per spec)
    Wqb = Wqb * gqa[:, None]
    Wkvb = Wkvb * gkva[:, None]

    # sign pattern for the shuffle-based rotate_half
    sign = np.where(DIMS_PERM < RD // 2, -1.0, 1.0).astype(np.float32)[:, None]

    tri = np.zeros((128, 128), np.float32)
    kp, q = np.mgrid[0:128, 0:128]
    tri[q >= kp] = 1.0

    per_core = []
    w4 = Wqb.reshape(QL, NH, QHD)
    wk4 = Wkvb.reshape(KVL, NH, ND + VD)
    for c in range(NCORES):
        b, t = divmod(c, TP)
        heads = slice(t * HPC, (t + 1) * HPC)
        # Wqb: group-blocked [h0 nope | h1 nope | h0 pe' | h1 pe'] per group
        wq = w4[:, heads]                       # (QL, 16, 192)
        nope = wq[:, :, :ND]                    # (QL, 16, 128)
        pe = wq[:, :, ND:][:, :, DIMS_PERM]     # (QL, 16, 64) permuted
        blocks = []
        for g in range(NG):
            blocks.extend([nope[:, 2 * g], nope[:, 2 * g + 1], pe[:, 2 * g], pe[:, 2 * g + 1]])
        wqb_c = np.ascontiguousarray(np.concatenate(blocks, axis=1))

        wkc = wk4[:, heads]
        wkvbk_c = np.ascontiguousarray(wkc[:, :, :ND].reshape(KVL, HPC * ND))
        wkvbv_c = np.ascontiguousarray(wkc[:, :, ND:].reshape(KVL, HPC * VD))
        wo_c = np.ascontiguousarray(Wo[t * HPC * VD:(t + 1) * HPC * VD])

        cos_g = cos[pid[b]]                     # (S, RD)
        sin_g = sin[pid[b]]
        cosT = np.ascontiguousarray(cos_g.T[DIMS_PERM])   # (64, S)
        sinT = np.ascontiguousarray(sin_g.T[DIMS_PERM])
        csq = np.ascontiguousarray(np.vstack([cosT, cosT]))
        ssq = np.ascontiguousarray(np.vstack([sinT * sign, sinT * sign]))

        per_core.append({
            "hsT": np.ascontiguousarray(hs[b].T),
            "wa": wa,
            "wqb": wqb_c,
            "wkvbk": wkvbk_c,
            "wkvbv": wkvbv_c,
            "wo": wo_c,
            "csq": csq,
            "ssq": ssq,
            "tri": tri,
            "ones_in": np.ones((128, 1), np.float32),
        })
    return per_core


def kernel(**inputs):
    if "nc" not in _NC_CACHE:
        _NC_CACHE["nc"] = _build_nc()
    nc = _NC_CACHE["nc"]
    in_maps = _host_prep(inputs)
    res = bass_utils.run_bass_kernel_spmd(nc, in_maps, core_ids=list(range(NCORES)))
    outs = []
    for b in range(B):
        acc = res.results[TP * b]["outT"].astype(np.float32)
        for t in range(1, TP):
            acc = acc + res.results[TP * b + t]["outT"]
        outs.append(acc.T)
    return np.stack(outs, axis=0)
